# revision 45
# baseline (speedup 1.0000x reference)
"""NeRF MLP forward (nn_NeRFOriginal) on 8 Trainium2 NeuronCores.

Strategy
--------
Pure data parallel over the N=262144 sample dim (32768 samples/core), weights
replicated. On each core the activations live "channel-on-partition": a sample
tile is [C<=128 partitions, T=512 samples] so every layer is
    psum[m_out_block, T] += w[k_block, m_block].T @ y[k_block, T]
with the weight slice as the stationary operand and samples streaming on the
free dim (N=512 = one fp32 PSUM bank). All matmul operands are fp16
(1 PE cycle/row vs 2.0 for fp32; measured end-to-end l2 rel-err ~3.5e-4),
accumulation is fp32 in PSUM. Bias+ReLU are fused ops on ScalarE/VectorE
(alternating, to balance), reading PSUM and writing the next fp16 activation
tile. Layer-0 / layer-5(skip) / views-head biases are folded into the matmul
via a ones-row in the input tile and a bias-row in the weight pack.

Host side packs: x -> xT [96, 32768] fp16 per core (pos ch 0..62, ones row 63,
view ch rows 64..90, ones row 91), all weights -> one [128, CW] fp16 pack, all
remaining biases -> one [128, NB] fp32 pack. Outputs come back channel-major
(rgbT [3,S], alphaT [1,S]) and are re-assembled on host.
"""

import numpy as np

N = 262144
IN_CH = 63
IN_CHV = 27
W = 256
NCORES = 8
S = N // NCORES  # 32768 samples per core
T = 512          # samples per tile (one fp32 PSUM bank)
NT = S // T      # 64 tiles/core

F16 = np.float16
F32 = np.float32


# --------------------------------------------------------------------------
# Static layout of the weight/bias packs (shared by host packing + emission)
# --------------------------------------------------------------------------
def _wlayout():
    """block key -> (p0, K, c0, M): rows p0:p0+K, cols c0:c0+M of the pack.

    Every block is a full 128-row tile (ragged K zero-padded): non-128x128
    LDWEIGHTS can't use the PE background weight buffer and exposes ~95ns
    per matmul. xt rows beyond the data are zeros, so padding is exact.
    """
    off = {}
    c = 0
    for m in range(2):                      # L0: pos(63)+ones, rows 64.. zero
        off[("L0", "pos", m)] = (0, 128, c, 128)
        c += 128
    for i in (1, 2, 3, 4, 6, 7):            # plain 256->256 layers
        for k in range(2):
            for m in range(2):
                off[("L", i, k, m)] = (0, 128, c, 128)
                c += 128
    for ks in ("pos", 0, 1):                # L5: [pos(63)+ones; y4] -> 256
        for m in range(2):
            off[("L", 5, ks, m)] = (0, 128, c, 128)
            c += 128
    for k in range(2):                      # feature head 256->256
        for m in range(2):
            off[("feat", k, m)] = (0, 128, c, 128)
            c += 128
    for k in range(2):                      # alpha head 256->1 (M zero-padded
        off[("alpha", k)] = (0, 128, c, 128)  # to 128 for full-tile LDWEIGHTS)
        c += 128
    off[("views", 0)] = (0, 128, c, 128)    # views head 283->128
    c += 128
    off[("views", 1)] = (0, 128, c, 128)
    c += 128
    off[("views", "v")] = (0, 128, c, 128)  # view dirs at rows 64..90 + ones@91
    c += 128
    off[("rgb",)] = (0, 128, c, 3)          # rgb head 128->3
    c += 3
    return off, c


def _blayout():
    off = {}
    c = 0
    for i in (1, 2, 3, 4, 6, 7):
        for m in range(2):
            off[("b", i, m)] = c
            c += 1
    for m in range(2):
        off[("bfeat", m)] = c
        c += 1
    off[("balpha",)] = c
    c += 1
    off[("brgb",)] = c
    c += 1
    return off, c


WOFF, _CW_RAW = _wlayout()
CW = (_CW_RAW + 63) & ~63     # pad cols
BOFF, NB = _blayout()


# --------------------------------------------------------------------------
# Host-side packing
# --------------------------------------------------------------------------
def _pack_weights(inp):
    wp = np.zeros((128, CW), dtype=F16)
    bp = np.zeros((128, NB), dtype=F32)
    ws = [inp[f"w{i}"] for i in range(8)]
    bs = [inp[f"b{i}"] for i in range(8)]

    def put(key, arr, row0=0):
        p0, K, c0, M = WOFF[key]
        h, w_ = arr.shape
        assert row0 + h <= K and w_ <= M, (key, arr.shape, (K, M))
        wp[p0 + row0 : p0 + row0 + h, c0 : c0 + w_] = arr.astype(F16)

    def put_bias_row(key, row, vec):
        p0, K, c0, M = WOFF[key]
        wp[p0 + row, c0 : c0 + M] = vec.astype(F16)

    for m in range(2):
        sl = slice(m * 128, (m + 1) * 128)
        put(("L0", "pos", m), ws[0][:, sl])            # rows 0:63
        put_bias_row(("L0", "pos", m), 63, bs[0][sl])  # ones-row bias
        for i in (1, 2, 3, 4, 6, 7):
            for k in range(2):
                put(("L", i, k, m), ws[i][k * 128 : (k + 1) * 128, sl])
            bp[:, BOFF[("b", i, m)]] = bs[i][sl].astype(F32)
        put(("L", 5, "pos", m), ws[5][0:63, sl])
        put_bias_row(("L", 5, "pos", m), 63, bs[5][sl])
        put(("L", 5, 0, m), ws[5][63:191, sl])
        put(("L", 5, 1, m), ws[5][191:319, sl])
        for k in range(2):
            put(("feat", k, m), inp["feat_w"][k * 128 : (k + 1) * 128, sl])
        bp[:, BOFF[("bfeat", m)]] = inp["feat_b"][sl].astype(F32)

    for k in range(2):
        put(("alpha", k), inp["alpha_w"][k * 128 : (k + 1) * 128, :])  # col 0 only
        put(("views", k), inp["views_w"][k * 128 : (k + 1) * 128, :])
    put(("views", "v"), inp["views_w"][256:283, :], row0=64)  # rows 64:91
    put_bias_row(("views", "v"), 91, inp["views_b"])          # partition 91
    put(("rgb",), inp["rgb_w"])
    bp[0, BOFF[("balpha",)]] = float(inp["alpha_b"][0])
    bp[0:3, BOFF[("brgb",)]] = inp["rgb_b"].astype(F32)
    return wp, bp


def _pack_x(x_shard):
    """[S, 90] fp32 -> [128, S] fp16: pos rows 0:63, ones row 63, view rows
    64:91, ones row 91, zeros above (so full-128-row matmuls are exact)."""
    s = x_shard.shape[0]
    xp = np.zeros((128, s), dtype=F16)
    xp[0:63, :] = np.ascontiguousarray(x_shard[:, 0:63].T).astype(F16)
    xp[63, :] = 1.0
    xp[64:91, :] = np.ascontiguousarray(x_shard[:, 63:90].T).astype(F16)
    xp[91, :] = 1.0
    return xp


# --------------------------------------------------------------------------
# Device program
# --------------------------------------------------------------------------
def _emit(nc, tc, aps, n_tiles):
    """Emit the whole per-core program under TileContext tc."""
    import concourse.mybir as mybir

    f16 = mybir.dt.float16
    f32 = mybir.dt.float32
    RELU = mybir.ActivationFunctionType.Relu
    IDENT = mybir.ActivationFunctionType.Identity
    ADD = mybir.AluOpType.add
    MAX = mybir.AluOpType.max

    xT, WP, BP, rgbT, alphaT = aps

    from contextlib import ExitStack

    with ExitStack() as ctx:
        wpool = ctx.enter_context(tc.tile_pool(name="w", bufs=1))
        xpool = ctx.enter_context(tc.tile_pool(name="x", bufs=8))
        ypool = ctx.enter_context(tc.tile_pool(name="y", bufs=12))
        fpool = ctx.enter_context(tc.tile_pool(name="f", bufs=8))
        opool = ctx.enter_context(tc.tile_pool(name="o", bufs=10))
        pspool = ctx.enter_context(tc.tile_pool(name="ps", bufs=8, space="PSUM"))

        wt = wpool.tile([128, CW], f16, tag="wp")
        bt = wpool.tile([128, NB], f32, tag="bp")

        def load_weights():
            # chunked pack load: layer-0/1 weights land in the first chunk,
            # and the first x tiles are DMA'd between chunks, so the first
            # matmuls start as early as possible
            NCHUNK = 8
            step = ((CW // NCHUNK) + 63) & ~63
            nc.sync.dma_start(out=wt[:, 0:step], in_=WP[:, 0:step])
            yield
            for c0 in range(step, CW, step):
                c1 = min(c0 + step, CW)
                nc.sync.dma_start(out=wt[:, c0:c1], in_=WP[:, c0:c1])
            nc.sync.dma_start(out=bt[:, :], in_=BP[:, :])

        def lhsT(key):
            p0, K, c0, M = WOFF[key]
            return wt[p0 : p0 + K, c0 : c0 + M]

        def bias(key, nrows=128, row0=0):
            c = BOFF[key]
            return bt[row0 : row0 + nrows, c : c + 1]

        def mm_group(ps_ap, pairs):
            """accumulate sum_k lhsT(key_k).T @ rhs_k into ps_ap"""
            n = len(pairs)
            for j, (key, rhs) in enumerate(pairs):
                nc.tensor.matmul(
                    ps_ap,
                    lhsT=lhsT(key),
                    rhs=rhs,
                    start=(j == 0),
                    stop=(j == n - 1),
                )

        def ew(par, out_ap, in_ap, b_ap, relu):
            """fused (x + b) / relu, alternating ScalarE / VectorE.
            par = (tile_index + per-tile op counter) % 2 so that the two
            engines stay balanced and a layer's two blocks run in parallel."""
            use_act = par % 2 == 0
            if use_act:
                nc.scalar.activation(
                    out=out_ap,
                    in_=in_ap,
                    func=RELU if relu else IDENT,
                    bias=b_ap if b_ap is not None else 0.0,
                )
            else:
                if b_ap is not None and relu:
                    nc.vector.tensor_scalar(
                        out=out_ap, in0=in_ap, scalar1=b_ap, scalar2=0.0,
                        op0=ADD, op1=MAX,
                    )
                elif relu:
                    nc.vector.tensor_scalar_max(out_ap, in_ap, 0.0)
                else:
                    nc.vector.tensor_scalar_add(out_ap, in_ap, b_ap)

        xts = {}

        def load_x(t):
            if t >= n_tiles:
                return
            xt = xpool.tile([128, T], f16, tag="xt")
            nc.sync.dma_start(out=xt[:, :], in_=xT[:, t * T : (t + 1) * T])
            xts[t] = xt

        y0s = {}

        def emit_l0(t):
            """L0 for tile t, emitted during the PREVIOUS pair's tail so its
            eltwise is long done before this tile's L1 matmuls issue."""
            if t >= n_tiles:
                return
            xt = xts[t]
            y = ypool.tile([128, 2 * T], f16, tag="y")
            for m in range(2):
                ps = pspool.tile([128, T], f32, tag="ps")
                mm_group(ps[:, :], [(("L0", "pos", m), xt[:, :])])
                ew(t + 1 + m, y[:, m * T : (m + 1) * T], ps[:, :], None, True)
            y0s[t] = y

        def tile_gen(t):
            c0 = t * T
            op = [t]  # per-tile eltwise engine counter (balanced across tiles)

            def nxt():
                op[0] += 1
                return op[0]

            xt = xts.pop(t)
            y = y0s.pop(t)
            op[0] = t + 2  # L0's two eltwise ops used t+1, t+2

            y4 = None
            for i in range(1, 8):
                yn = ypool.tile([128, 2 * T], f16, tag="y")
                for m in range(2):
                    ps = pspool.tile([128, T], f32, tag="ps")
                    if i == 5:
                        mm_group(
                            ps[:, :],
                            [
                                (("L", 5, "pos", m), xt[:, :]),
                                (("L", 5, 0, m), y4[:, 0:T]),
                                (("L", 5, 1, m), y4[:, T : 2 * T]),
                            ],
                        )
                        ew(nxt(), yn[:, m * T : (m + 1) * T], ps[:, :], None, True)
                    else:
                        mm_group(
                            ps[:, :],
                            [
                                (("L", i, 0, m), y[:, 0:T]),
                                (("L", i, 1, m), y[:, T : 2 * T]),
                            ],
                        )
                        ew(
                            nxt(),
                            yn[:, m * T : (m + 1) * T],
                            ps[:, :],
                            bias(("b", i, m)),
                            True,
                        )
                if i == 4:
                    y4 = yn  # layer-4 output, consumed by the skip at L5
                y = yn
                yield

            # feature head (no relu)
            feat = fpool.tile([128, 2 * T], f16, tag="feat")
            for m in range(2):
                ps = pspool.tile([128, T], f32, tag="ps")
                mm_group(
                    ps[:, :],
                    [(("feat", 0, m), y[:, 0:T]), (("feat", 1, m), y[:, T : 2 * T])],
                )
                ew(nxt(), feat[:, m * T : (m + 1) * T], ps[:, :], bias(("bfeat", m)), False)
            yield

            # views head (bias folded via ones row 91; pos rows hit zero weights)
            psv = pspool.tile([128, T], f32, tag="ps")
            mm_group(
                psv[:, :],
                [
                    (("views", 0), feat[:, 0:T]),
                    (("views", 1), feat[:, T : 2 * T]),
                    (("views", "v"), xt[:, :]),
                ],
            )
            h = fpool.tile([128, T], f16, tag="h")
            ew(nxt(), h[:, :], psv[:, :], None, True)
            yield

            # alpha head (also serves as PE cover for the h eltwise);
            # M is zero-padded to 128 so only psum row 0 is meaningful
            psa = pspool.tile([128, T], f32, tag="ps")
            mm_group(
                psa[:, :],
                [(("alpha", 0), y[:, 0:T]), (("alpha", 1), y[:, T : 2 * T])],
            )
            oa = opool.tile([1, T], f32, tag="oa")
            ew(nxt(), oa[:, :], psa[0:1, :], bias(("balpha",), 1), False)
            nc.sync.dma_start(out=alphaT[:, c0 : c0 + T], in_=oa[:, :])
            yield

            # rgb head
            psr = pspool.tile([3, T], f32, tag="ps")
            mm_group(psr[:, :], [(("rgb",), h[:, :])])
            orgb = opool.tile([3, T], f32, tag="orgb")
            ew(nxt(), orgb[:, :], psr[:, :], bias(("brgb",), 3), False)
            nc.sync.dma_start(out=rgbT[:, c0 : c0 + T], in_=orgb[:, :])
            yield

        # Software-pipeline tiles in groups of 4; within a phase the other
        # three tiles' matmuls (~12 MMs, ~2.6us) cover each tile's eltwise
        # latency. x loads are issued one group ahead, and the next group's
        # L0 is emitted inside this group's tail (after views) so the L0
        # relus are complete before the next group's L1 matmuls reach PE.
        GROUP = 4
        lw = load_weights()
        next(lw)          # chunk 0 (layers 0-4)
        for t in range(min(GROUP, n_tiles)):
            load_x(t)
        next(lw, None)    # remaining chunks + biases
        for t in range(min(GROUP, n_tiles)):
            emit_l0(t)
        NPH = 11  # tile_gen phases: L1..L7, feat, views, alpha, rgb
        VIEWS_PH = 9
        for tg in range(0, n_tiles, GROUP):
            for t in range(tg + GROUP, min(tg + 2 * GROUP, n_tiles)):
                load_x(t)
            gens = [tile_gen(t) for t in range(tg, min(tg + GROUP, n_tiles))]
            for ph in range(1, NPH + 1):
                for g in gens:
                    next(g, None)
                if ph == VIEWS_PH:
                    for t in range(tg + GROUP, min(tg + 2 * GROUP, n_tiles)):
                        emit_l0(t)


def build_program(n_samples=S):
    """Build + compile the per-core Bass program. Returns the Bacc object."""
    import concourse.mybir as mybir
    import concourse.tile as tile
    from concourse import bacc

    n_tiles = n_samples // T
    nc = bacc.Bacc(
        "TRN2",
        target_bir_lowering=False,
        debug=False,
        enable_asserts=False,
        num_devices=NCORES,
    )
    f16 = mybir.dt.float16
    f32 = mybir.dt.float32
    xT = nc.dram_tensor("xt", [128, n_samples], f16, kind="ExternalInput").ap()
    WP = nc.dram_tensor("wp", [128, CW], f16, kind="ExternalInput").ap()
    BP = nc.dram_tensor("bp", [128, NB], f32, kind="ExternalInput").ap()
    rgbT = nc.dram_tensor("rgbT", [3, n_samples], f32, kind="ExternalOutput").ap()
    alphaT = nc.dram_tensor("alphaT", [1, n_samples], f32, kind="ExternalOutput").ap()

    with tile.TileContext(nc) as tc:
        _emit(nc, tc, (xT, WP, BP, rgbT, alphaT), n_tiles)
    nc.compile()
    return nc


_PROG = None


def _program():
    global _PROG
    if _PROG is None:
        _PROG = build_program()
    return _PROG


def run(inputs, trace=False, **spmd_kwargs):
    """Shard, run on 8 cores, gather. Returns ((out[N,4], zeros[N,3]), results)."""
    from concourse import bass_utils

    nc = _program()
    x = np.asarray(inputs["x"], dtype=F32)
    wp, bp = _pack_weights(inputs)
    in_maps = []
    for c in range(NCORES):
        shard = x[c * S : (c + 1) * S]
        in_maps.append({"xt": _pack_x(shard), "wp": wp, "bp": bp})

    res = bass_utils.run_bass_kernel_spmd(
        nc, in_maps, core_ids=list(range(NCORES)), trace=trace, **spmd_kwargs
    )

    out = np.empty((N, 4), dtype=F32)
    for c, r in enumerate(res.results):
        sl = slice(c * S, (c + 1) * S)
        out[sl, 0:3] = r["rgbT"].T
        out[sl, 3] = r["alphaT"][0]
    zeros = np.zeros((N, 3), dtype=F32)
    return (out, zeros), res


def kernel(**inputs):
    (out, zeros), _ = run(inputs)
    return (out, zeros)


# revision 46
# speedup vs baseline: 1.0050x; 1.0050x over previous
"""NeRF MLP forward (nn_NeRFOriginal) on 8 Trainium2 NeuronCores.

Strategy
--------
Pure data parallel over the N=262144 sample dim (32768 samples/core), weights
replicated. On each core the activations live "channel-on-partition": a sample
tile is [C<=128 partitions, T=512 samples] so every layer is
    psum[m_out_block, T] += w[k_block, m_block].T @ y[k_block, T]
with the weight slice as the stationary operand and samples streaming on the
free dim (N=512 = one fp32 PSUM bank). All matmul operands are fp16
(1 PE cycle/row vs 2.0 for fp32; measured end-to-end l2 rel-err ~3.5e-4),
accumulation is fp32 in PSUM. Bias+ReLU are fused ops on ScalarE/VectorE
(alternating, to balance), reading PSUM and writing the next fp16 activation
tile. Layer-0 / layer-5(skip) / views-head biases are folded into the matmul
via a ones-row in the input tile and a bias-row in the weight pack.

Host side packs: x -> xT [96, 32768] fp16 per core (pos ch 0..62, ones row 63,
view ch rows 64..90, ones row 91), all weights -> one [128, CW] fp16 pack, all
remaining biases -> one [128, NB] fp32 pack. Outputs come back channel-major
(rgbT [3,S], alphaT [1,S]) and are re-assembled on host.
"""

import numpy as np

N = 262144
IN_CH = 63
IN_CHV = 27
W = 256
NCORES = 8
S = N // NCORES  # 32768 samples per core
T = 512          # samples per tile (one fp32 PSUM bank)
NT = S // T      # 64 tiles/core

F16 = np.float16
F32 = np.float32


# --------------------------------------------------------------------------
# Static layout of the weight/bias packs (shared by host packing + emission)
# --------------------------------------------------------------------------
def _wlayout():
    """block key -> (p0, K, c0, M): rows p0:p0+K, cols c0:c0+M of the pack.

    Every block is a full 128-row tile (ragged K zero-padded): non-128x128
    LDWEIGHTS can't use the PE background weight buffer and exposes ~95ns
    per matmul. xt rows beyond the data are zeros, so padding is exact.
    """
    off = {}
    c = 0
    for m in range(2):                      # L0: pos(63)+ones, rows 64.. zero
        off[("L0", "pos", m)] = (0, 128, c, 128)
        c += 128
    for i in (1, 2, 3, 4, 6, 7):            # plain 256->256 layers
        for k in range(2):
            for m in range(2):
                off[("L", i, k, m)] = (0, 128, c, 128)
                c += 128
    for ks in ("pos", 0, 1):                # L5: [pos(63)+ones; y4] -> 256
        for m in range(2):
            off[("L", 5, ks, m)] = (0, 128, c, 128)
            c += 128
    for k in range(2):                      # feature head 256->256
        for m in range(2):
            off[("feat", k, m)] = (0, 128, c, 128)
            c += 128
    for k in range(2):                      # alpha head 256->1 (M zero-padded
        off[("alpha", k)] = (0, 128, c, 128)  # to 128 for full-tile LDWEIGHTS)
        c += 128
    off[("views", 0)] = (0, 128, c, 128)    # views head 283->128
    c += 128
    off[("views", 1)] = (0, 128, c, 128)
    c += 128
    off[("views", "v")] = (0, 128, c, 128)  # view dirs at rows 64..90 + ones@91
    c += 128
    off[("rgb",)] = (0, 128, c, 3)          # rgb head 128->3
    c += 3
    return off, c


def _blayout():
    off = {}
    c = 0
    for i in (1, 2, 3, 4, 6, 7):
        for m in range(2):
            off[("b", i, m)] = c
            c += 1
    for m in range(2):
        off[("bfeat", m)] = c
        c += 1
    off[("balpha",)] = c
    c += 1
    off[("brgb",)] = c
    c += 1
    return off, c


WOFF, _CW_RAW = _wlayout()
CW = (_CW_RAW + 63) & ~63     # pad cols
BOFF, NB = _blayout()


# --------------------------------------------------------------------------
# Host-side packing
# --------------------------------------------------------------------------
def _pack_weights(inp):
    wp = np.zeros((128, CW), dtype=F16)
    bp = np.zeros((128, NB), dtype=F32)
    ws = [inp[f"w{i}"] for i in range(8)]
    bs = [inp[f"b{i}"] for i in range(8)]

    def put(key, arr, row0=0):
        p0, K, c0, M = WOFF[key]
        h, w_ = arr.shape
        assert row0 + h <= K and w_ <= M, (key, arr.shape, (K, M))
        wp[p0 + row0 : p0 + row0 + h, c0 : c0 + w_] = arr.astype(F16)

    def put_bias_row(key, row, vec):
        p0, K, c0, M = WOFF[key]
        wp[p0 + row, c0 : c0 + M] = vec.astype(F16)

    for m in range(2):
        sl = slice(m * 128, (m + 1) * 128)
        put(("L0", "pos", m), ws[0][:, sl])            # rows 0:63
        put_bias_row(("L0", "pos", m), 63, bs[0][sl])  # ones-row bias
        for i in (1, 2, 3, 4, 6, 7):
            for k in range(2):
                put(("L", i, k, m), ws[i][k * 128 : (k + 1) * 128, sl])
            bp[:, BOFF[("b", i, m)]] = bs[i][sl].astype(F32)
        put(("L", 5, "pos", m), ws[5][0:63, sl])
        put_bias_row(("L", 5, "pos", m), 63, bs[5][sl])
        put(("L", 5, 0, m), ws[5][63:191, sl])
        put(("L", 5, 1, m), ws[5][191:319, sl])
        for k in range(2):
            put(("feat", k, m), inp["feat_w"][k * 128 : (k + 1) * 128, sl])
        bp[:, BOFF[("bfeat", m)]] = inp["feat_b"][sl].astype(F32)

    for k in range(2):
        put(("alpha", k), inp["alpha_w"][k * 128 : (k + 1) * 128, :])  # col 0 only
        put(("views", k), inp["views_w"][k * 128 : (k + 1) * 128, :])
    put(("views", "v"), inp["views_w"][256:283, :], row0=64)  # rows 64:91
    put_bias_row(("views", "v"), 91, inp["views_b"])          # partition 91
    put(("rgb",), inp["rgb_w"])
    bp[0, BOFF[("balpha",)]] = float(inp["alpha_b"][0])
    bp[0:3, BOFF[("brgb",)]] = inp["rgb_b"].astype(F32)
    return wp, bp


def _pack_x(x_shard):
    """[S, 90] fp32 -> [128, S] fp16: pos rows 0:63, ones row 63, view rows
    64:91, ones row 91, zeros above (so full-128-row matmuls are exact)."""
    s = x_shard.shape[0]
    xp = np.zeros((128, s), dtype=F16)
    xp[0:63, :] = np.ascontiguousarray(x_shard[:, 0:63].T).astype(F16)
    xp[63, :] = 1.0
    xp[64:91, :] = np.ascontiguousarray(x_shard[:, 63:90].T).astype(F16)
    xp[91, :] = 1.0
    return xp


# --------------------------------------------------------------------------
# Device program
# --------------------------------------------------------------------------
def _emit(nc, tc, aps, n_tiles):
    """Emit the whole per-core program under TileContext tc."""
    import concourse.mybir as mybir

    f16 = mybir.dt.float16
    f32 = mybir.dt.float32
    RELU = mybir.ActivationFunctionType.Relu
    IDENT = mybir.ActivationFunctionType.Identity
    ADD = mybir.AluOpType.add
    MAX = mybir.AluOpType.max

    xT, WP, BP, rgbT, alphaT = aps

    from contextlib import ExitStack

    with ExitStack() as ctx:
        wpool = ctx.enter_context(tc.tile_pool(name="w", bufs=1))
        xpool = ctx.enter_context(tc.tile_pool(name="x", bufs=8))
        ypool = ctx.enter_context(tc.tile_pool(name="y", bufs=12))
        fpool = ctx.enter_context(tc.tile_pool(name="f", bufs=8))
        opool = ctx.enter_context(tc.tile_pool(name="o", bufs=10))
        pspool = ctx.enter_context(tc.tile_pool(name="ps", bufs=8, space="PSUM"))

        wt = wpool.tile([128, CW], f16, tag="wp")
        bt = wpool.tile([128, NB], f32, tag="bp")

        def load_weights():
            # chunked pack load: layer-0/1 weights land in the first chunk,
            # and the first x tiles are DMA'd between chunks, so the first
            # matmuls start as early as possible
            NCHUNK = 4
            step = ((CW // NCHUNK) + 63) & ~63
            nc.sync.dma_start(out=wt[:, 0:step], in_=WP[:, 0:step])
            yield
            for c0 in range(step, CW, step):
                c1 = min(c0 + step, CW)
                nc.sync.dma_start(out=wt[:, c0:c1], in_=WP[:, c0:c1])
            nc.sync.dma_start(out=bt[:, :], in_=BP[:, :])

        def lhsT(key):
            p0, K, c0, M = WOFF[key]
            return wt[p0 : p0 + K, c0 : c0 + M]

        def bias(key, nrows=128, row0=0):
            c = BOFF[key]
            return bt[row0 : row0 + nrows, c : c + 1]

        def mm_group(ps_ap, pairs):
            """accumulate sum_k lhsT(key_k).T @ rhs_k into ps_ap"""
            n = len(pairs)
            for j, (key, rhs) in enumerate(pairs):
                nc.tensor.matmul(
                    ps_ap,
                    lhsT=lhsT(key),
                    rhs=rhs,
                    start=(j == 0),
                    stop=(j == n - 1),
                )

        def ew(par, out_ap, in_ap, b_ap, relu):
            """fused (x + b) / relu, alternating ScalarE / VectorE.
            par = (tile_index + per-tile op counter) % 2 so that the two
            engines stay balanced and a layer's two blocks run in parallel."""
            use_act = par % 2 == 0
            if use_act:
                nc.scalar.activation(
                    out=out_ap,
                    in_=in_ap,
                    func=RELU if relu else IDENT,
                    bias=b_ap if b_ap is not None else 0.0,
                )
            else:
                if b_ap is not None and relu:
                    nc.vector.tensor_scalar(
                        out=out_ap, in0=in_ap, scalar1=b_ap, scalar2=0.0,
                        op0=ADD, op1=MAX,
                    )
                elif relu:
                    nc.vector.tensor_scalar_max(out_ap, in_ap, 0.0)
                else:
                    nc.vector.tensor_scalar_add(out_ap, in_ap, b_ap)

        xts = {}

        def load_x(t):
            if t >= n_tiles:
                return
            xt = xpool.tile([128, T], f16, tag="xt")
            nc.sync.dma_start(out=xt[:, :], in_=xT[:, t * T : (t + 1) * T])
            xts[t] = xt

        y0s = {}

        def emit_l0(t):
            """L0 for tile t, emitted during the PREVIOUS pair's tail so its
            eltwise is long done before this tile's L1 matmuls issue."""
            if t >= n_tiles:
                return
            xt = xts[t]
            y = ypool.tile([128, 2 * T], f16, tag="y")
            for m in range(2):
                ps = pspool.tile([128, T], f32, tag="ps")
                mm_group(ps[:, :], [(("L0", "pos", m), xt[:, :])])
                ew(t + 1 + m, y[:, m * T : (m + 1) * T], ps[:, :], None, True)
            y0s[t] = y

        def tile_gen(t):
            c0 = t * T
            op = [t]  # per-tile eltwise engine counter (balanced across tiles)

            def nxt():
                op[0] += 1
                return op[0]

            xt = xts.pop(t)
            y = y0s.pop(t)
            op[0] = t + 2  # L0's two eltwise ops used t+1, t+2

            y4 = None
            for i in range(1, 8):
                yn = ypool.tile([128, 2 * T], f16, tag="y")
                for m in range(2):
                    ps = pspool.tile([128, T], f32, tag="ps")
                    if i == 5:
                        mm_group(
                            ps[:, :],
                            [
                                (("L", 5, "pos", m), xt[:, :]),
                                (("L", 5, 0, m), y4[:, 0:T]),
                                (("L", 5, 1, m), y4[:, T : 2 * T]),
                            ],
                        )
                        ew(nxt(), yn[:, m * T : (m + 1) * T], ps[:, :], None, True)
                    else:
                        mm_group(
                            ps[:, :],
                            [
                                (("L", i, 0, m), y[:, 0:T]),
                                (("L", i, 1, m), y[:, T : 2 * T]),
                            ],
                        )
                        ew(
                            nxt(),
                            yn[:, m * T : (m + 1) * T],
                            ps[:, :],
                            bias(("b", i, m)),
                            True,
                        )
                if i == 4:
                    y4 = yn  # layer-4 output, consumed by the skip at L5
                y = yn
                yield

            # feature head (no relu)
            feat = fpool.tile([128, 2 * T], f16, tag="feat")
            for m in range(2):
                ps = pspool.tile([128, T], f32, tag="ps")
                mm_group(
                    ps[:, :],
                    [(("feat", 0, m), y[:, 0:T]), (("feat", 1, m), y[:, T : 2 * T])],
                )
                ew(nxt(), feat[:, m * T : (m + 1) * T], ps[:, :], bias(("bfeat", m)), False)
            yield

            # views head (bias folded via ones row 91; pos rows hit zero weights)
            psv = pspool.tile([128, T], f32, tag="ps")
            mm_group(
                psv[:, :],
                [
                    (("views", 0), feat[:, 0:T]),
                    (("views", 1), feat[:, T : 2 * T]),
                    (("views", "v"), xt[:, :]),
                ],
            )
            h = fpool.tile([128, T], f16, tag="h")
            ew(nxt(), h[:, :], psv[:, :], None, True)
            yield

            # alpha head (also serves as PE cover for the h eltwise);
            # M is zero-padded to 128 so only psum row 0 is meaningful
            psa = pspool.tile([128, T], f32, tag="ps")
            mm_group(
                psa[:, :],
                [(("alpha", 0), y[:, 0:T]), (("alpha", 1), y[:, T : 2 * T])],
            )
            oa = opool.tile([1, T], f32, tag="oa")
            ew(nxt(), oa[:, :], psa[0:1, :], bias(("balpha",), 1), False)
            nc.sync.dma_start(out=alphaT[:, c0 : c0 + T], in_=oa[:, :])
            yield

            # rgb head
            psr = pspool.tile([3, T], f32, tag="ps")
            mm_group(psr[:, :], [(("rgb",), h[:, :])])
            orgb = opool.tile([3, T], f32, tag="orgb")
            ew(nxt(), orgb[:, :], psr[:, :], bias(("brgb",), 3), False)
            nc.sync.dma_start(out=rgbT[:, c0 : c0 + T], in_=orgb[:, :])
            yield

        # Software-pipeline tiles in groups of 4; within a phase the other
        # three tiles' matmuls (~12 MMs, ~2.6us) cover each tile's eltwise
        # latency. x loads are issued one group ahead, and the next group's
        # L0 is emitted inside this group's tail (after views) so the L0
        # relus are complete before the next group's L1 matmuls reach PE.
        GROUP = 4
        lw = load_weights()
        next(lw)          # chunk 0 (layers 0-4)
        for t in range(min(GROUP, n_tiles)):
            load_x(t)
        next(lw, None)    # remaining chunks + biases
        for t in range(min(GROUP, n_tiles)):
            emit_l0(t)
        NPH = 11  # tile_gen phases: L1..L7, feat, views, alpha, rgb
        VIEWS_PH = 9
        for tg in range(0, n_tiles, GROUP):
            for t in range(tg + GROUP, min(tg + 2 * GROUP, n_tiles)):
                load_x(t)
            gens = [tile_gen(t) for t in range(tg, min(tg + GROUP, n_tiles))]
            for ph in range(1, NPH + 1):
                for g in gens:
                    next(g, None)
                if ph == VIEWS_PH:
                    for t in range(tg + GROUP, min(tg + 2 * GROUP, n_tiles)):
                        emit_l0(t)


def build_program(n_samples=S):
    """Build + compile the per-core Bass program. Returns the Bacc object."""
    import concourse.mybir as mybir
    import concourse.tile as tile
    from concourse import bacc

    n_tiles = n_samples // T
    nc = bacc.Bacc(
        "TRN2",
        target_bir_lowering=False,
        debug=False,
        enable_asserts=False,
        num_devices=NCORES,
    )
    f16 = mybir.dt.float16
    f32 = mybir.dt.float32
    xT = nc.dram_tensor("xt", [128, n_samples], f16, kind="ExternalInput").ap()
    WP = nc.dram_tensor("wp", [128, CW], f16, kind="ExternalInput").ap()
    BP = nc.dram_tensor("bp", [128, NB], f32, kind="ExternalInput").ap()
    rgbT = nc.dram_tensor("rgbT", [3, n_samples], f32, kind="ExternalOutput").ap()
    alphaT = nc.dram_tensor("alphaT", [1, n_samples], f32, kind="ExternalOutput").ap()

    with tile.TileContext(nc) as tc:
        _emit(nc, tc, (xT, WP, BP, rgbT, alphaT), n_tiles)
    nc.compile()
    return nc


_PROG = None


def _program():
    global _PROG
    if _PROG is None:
        _PROG = build_program()
    return _PROG


def run(inputs, trace=False, **spmd_kwargs):
    """Shard, run on 8 cores, gather. Returns ((out[N,4], zeros[N,3]), results)."""
    from concourse import bass_utils

    nc = _program()
    x = np.asarray(inputs["x"], dtype=F32)
    wp, bp = _pack_weights(inputs)
    in_maps = []
    for c in range(NCORES):
        shard = x[c * S : (c + 1) * S]
        in_maps.append({"xt": _pack_x(shard), "wp": wp, "bp": bp})

    res = bass_utils.run_bass_kernel_spmd(
        nc, in_maps, core_ids=list(range(NCORES)), trace=trace, **spmd_kwargs
    )

    out = np.empty((N, 4), dtype=F32)
    for c, r in enumerate(res.results):
        sl = slice(c * S, (c + 1) * S)
        out[sl, 0:3] = r["rgbT"].T
        out[sl, 3] = r["alphaT"][0]
    zeros = np.zeros((N, 3), dtype=F32)
    return (out, zeros), res


def kernel(**inputs):
    (out, zeros), _ = run(inputs)
    return (out, zeros)
